# revision 15
# baseline (speedup 1.0000x reference)
"""Dot-product attention TRN2 Bass kernel (v6: bf16, row-tiled QK, paired-FD
exp on a 3-slot score buffer).

Full inputs: queries/keys/values [32, 2048, 64] fp32.
Sharding: 32 heads split across 8 NeuronCores (4 heads each), no communication.

Per-core schedule (heads processed as 2 pairs; all matmul data bf16, fp32 PSUM):
  1. SWDGE cast-DMAs load Q/K/V per pair as bf16 into [128q, 16t, (2h x 64d)].
  2. Q^T/K^T [128(=2h x 64d), 2048q] built with 32 PE transposes/pair packed
     8-per-bank; the scratch bank is ALIASED into score slot 2 (bitcast view),
     and the next step's preprocessing chunks ride in PE/DVE slack at q-chunk
     boundaries of the current step's ACT-bound main loop.
  3. V|ones staged per head as [128k, 16t, 68] (col 64 = ones -> softmax
     denominator comes free out of the AV matmul).
  4. One flat unit stream over (step=rep x pair, q-chunk of 512, k-tile),
     global unit u, scores in a static PSUM tile [128, 3, 1024] (6 banks),
     slot = u % 3:
       S^T halves = row-tiled concurrent matmul pair -> score slot.
       exp(S^T/8): at u%3==1 ONE ACTIVATE FD=2048 covers slots 0-1 (two
       units); at u%3==2 an FD=1024 ACTIVATE covers slot 2 -> amortizes the
       ~220ns per-ACTIVATE overhead of the bottleneck engine.
       AV: O[128q, 65] += P^T-slice (stationary) @ V|ones, 8 small matmuls,
       emitted behind ACT via a global pending queue.
  5. Normalize straight from PSUM (reciprocal of denominator column +
     broadcast multiply) into fp32 staging; one DMA out per head.
PSUM budget: scores 6 banks (slot 2 doubles as transpose scratch) + accum 2.
No max-subtraction: scores ~N(0,1), exp safe in fp32.
"""
import sys

sys.path.insert(0, "/opt/trn_rl_repo")

from contextlib import ExitStack

import numpy as np

import concourse.bass as bass
import concourse.tile as tile
from concourse import bacc, mybir
from concourse.bass_utils import run_bass_kernel_spmd
from concourse.masks import make_identity

F32 = mybir.dt.float32
BF16 = mybir.dt.bfloat16
AF = mybir.ActivationFunctionType

N_CORES = 8
H = 4
NP = 2
L = 2048
D = 64
NT = L // 128
QC = 512
NQC = L // QC
SCALE = 1.0 / 8.0

_NC_CACHE = None


def _build_nc(reps=1):
    nc = bacc.Bacc("TRN2", target_bir_lowering=False, debug=False)
    q_d = nc.dram_tensor("queries", [H, L, D], F32, kind="ExternalInput").ap()
    k_d = nc.dram_tensor("keys", [H, L, D], F32, kind="ExternalInput").ap()
    v_d = nc.dram_tensor("values", [H, L, D], F32, kind="ExternalInput").ap()
    o_d = nc.dram_tensor("out", [H, L, D], F32, kind="ExternalOutput").ap()

    with tile.TileContext(nc) as tc, ExitStack() as ctx:
        sing = ctx.enter_context(tc.tile_pool(name="sing", bufs=1))
        stage = ctx.enter_context(tc.tile_pool(name="stage", bufs=2))
        tpose = ctx.enter_context(tc.tile_pool(name="tpose", bufs=2))
        vpool = ctx.enter_context(tc.tile_pool(name="vpool", bufs=2))
        ptbp = ctx.enter_context(tc.tile_pool(name="ptbp", bufs=2))
        ptsp = ctx.enter_context(tc.tile_pool(name="ptsp", bufs=2))
        outp = ctx.enter_context(tc.tile_pool(name="outp", bufs=2))
        rcp = ctx.enter_context(tc.tile_pool(name="rcp", bufs=4))
        ssp = ctx.enter_context(tc.tile_pool(name="ssp", bufs=1, space="PSUM"))
        acc_ = ctx.enter_context(tc.tile_pool(name="acc", bufs=1, space="PSUM"))

        ident = sing.tile([128, 128], F32)
        make_identity(nc, ident)
        ident_b = sing.tile([128, 128], BF16)
        nc.vector.tensor_copy(ident_b, ident)

        # static score buffer: 3 slots of [128, 1024] fp32 (2 banks each).
        # The first bank of slot 2 doubles as bf16 transpose scratch; the
        # Tile address tracker serializes scratch use against slot-2 QK/ACT.
        ss = ssp.tile([128, 3, 1024], F32)
        scr_flat = ss[:, 2, 0:512].bitcast(BF16)  # [128, 1024] bf16 = 1 bank
        scr3 = scr_flat.rearrange("q (a b) -> q a b", b=128)

        class Step:
            def __init__(self, p):
                self.p = p
                self.qt2 = None
                self.kt2 = None
                self.vo = None
                self.os_h = None
                self.accs = [None, None]

        def preproc_chunks(st):
            p = st.p
            qs = stage.tile([128, NT, 128], BF16, tag="qstg", name="qs")
            ks = stage.tile([128, NT, 128], BF16, tag="kstg", name="ks")
            vs = stage.tile([128, NT, 128], BF16, tag="vstg", name="vs")
            for h in range(2):
                src = lambda t_d: t_d[2 * p + h].rearrange(
                    "(t q) d -> q t d", q=128
                )
                nc.gpsimd.dma_start(ks[:, :, h * 64 : (h + 1) * 64], src(k_d))
            for h in range(2):
                src = lambda t_d: t_d[2 * p + h].rearrange(
                    "(t q) d -> q t d", q=128
                )
                nc.gpsimd.dma_start(qs[:, :, h * 64 : (h + 1) * 64], src(q_d))
                nc.gpsimd.dma_start(vs[:, :, h * 64 : (h + 1) * 64], src(v_d))
            yield None
            # Tile requests deferred: pool rotation must not happen until the
            # previous-previous step's pending AV/normalize uses are emitted.
            st.qt2 = tpose.tile([128, L], BF16, tag="qt", name="qt2")
            st.kt2 = tpose.tile([128, L], BF16, tag="kt", name="kt2")
            st.vo = vpool.tile([128, NT, 2, 68], BF16, tag="vo", name="vo")
            st.os_h = [
                outp.tile([128, NT, D], F32, tag=f"os{h}", name=f"os{h}")
                for h in range(2)
            ]
            for dst, stg, g in (
                (st.kt2, ks, 0),
                (st.qt2, qs, 0),
                (st.kt2, ks, 1),
                (st.qt2, qs, 1),
            ):
                for j in range(8):
                    nc.tensor.transpose(scr3[:, j, :], stg[:, g * 8 + j, :], ident_b)
                nc.vector.tensor_copy(
                    dst[:, g * 1024 : (g + 1) * 1024], scr_flat
                )
                yield None
            for h in range(2):
                nc.vector.tensor_copy(
                    st.vo[:, :, h, 0:64], vs[:, :, h * 64 : (h + 1) * 64]
                )
            nc.vector.memset(st.vo[:, :, :, 64:65], 1.0)
            yield None

        def flush(pend):
            st, qc, kt, pt = pend
            if kt == 0:
                st.accs = [
                    acc_.tile([128, 512], F32, tag=f"o{h}", name=f"o{h}")
                    for h in range(2)
                ]
            for h in range(2):
                for j in range(4):
                    # start=True clears has_written for the WHOLE bank: only
                    # the first matmul into each accumulator bank may set it.
                    nc.tensor.matmul(
                        st.accs[h][:, j * 66 : j * 66 + 65],
                        pt[:, (h * 4 + j) * 128 : (h * 4 + j + 1) * 128],
                        st.vo[:, kt, h, 0:65],
                        start=(kt == 0 and j == 0),
                        stop=(kt == NT - 1),
                    )
            if kt == NT - 1:
                for h in range(2):
                    av = st.accs[h][:, 0:264].rearrange("q (j c) -> q j c", c=66)
                    rc = rcp.tile([128, 4, 1], F32, tag="rc", name="rc")
                    nc.vector.reciprocal(rc, av[:, :, 64:65])
                    nc.vector.tensor_mul(
                        st.os_h[h][:, qc * 4 : (qc + 1) * 4, :],
                        av[:, :, 0:64],
                        rc.to_broadcast([128, 4, 64]),
                    )
                if qc == NQC - 1:
                    for h in range(2):
                        nc.sync.dma_start(
                            o_d[2 * st.p + h].rearrange("(t q) d -> q t d", q=128),
                            st.os_h[h],
                        )

        # ---- flat unit stream over (rep, pair, q-chunk, k-tile) ----
        steps = [Step(p) for _ in range(reps) for p in range(NP)]
        gen0 = preproc_chunks(steps[0])
        for _ in gen0:
            pass
        pending = []
        held = None
        ngen = None
        u = 0
        for i, st in enumerate(steps):
            if i + 1 < len(steps):
                ngen = preproc_chunks(steps[i + 1])
                next(ngen)  # prefetch the next step's input DMAs
            else:
                ngen = None
            for qc in range(NQC):
                for kt in range(NT):
                    slot = u % 3
                    for h in range(2):
                        nc.tensor.matmul(
                            ss[:, slot, h * 512 : (h + 1) * 512],
                            st.kt2[
                                h * 64 : (h + 1) * 64, kt * 128 : (kt + 1) * 128
                            ],
                            st.qt2[
                                h * 64 : (h + 1) * 64, qc * QC : (qc + 1) * QC
                            ],
                            start=True,
                            stop=True,
                        )
                    if pending:
                        flush(pending.pop(0))
                    if slot == 0:
                        held = (st, qc, kt)
                    elif slot == 1:
                        pt = ptbp.tile([128, 2048], BF16, tag="ptb", name="ptb")
                        nc.scalar.activation(pt, ss[:, 0:2, :], AF.Exp, scale=SCALE)
                        hst, hqc, hkt = held
                        pending.append((hst, hqc, hkt, pt[:, 0:1024]))
                        pending.append((st, qc, kt, pt[:, 1024:2048]))
                        held = None
                    else:
                        pt = ptsp.tile([128, 1024], BF16, tag="pts", name="pts")
                        nc.scalar.activation(pt, ss[:, 2, :], AF.Exp, scale=SCALE)
                        pending.append((st, qc, kt, pt))
                    u += 1
                if ngen is not None:
                    next(ngen, None)
            if ngen is not None:
                for _ in ngen:
                    pass
        if held is not None:
            st_, qc_, kt_ = held
            pt = ptsp.tile([128, 1024], BF16, tag="pts", name="pts")
            nc.scalar.activation(pt, ss[:, 0, :], AF.Exp, scale=SCALE)
            pending.append((st_, qc_, kt_, pt))
        while pending:
            flush(pending.pop(0))

    nc.compile()
    return nc


def _get_nc():
    global _NC_CACHE
    if _NC_CACHE is None:
        _NC_CACHE = _build_nc()
    return _NC_CACHE


def kernel(queries, keys, values):
    queries = np.ascontiguousarray(queries, dtype=np.float32)
    keys = np.ascontiguousarray(keys, dtype=np.float32)
    values = np.ascontiguousarray(values, dtype=np.float32)
    nc = _get_nc()
    in_maps = [
        {
            "queries": queries[c * H : (c + 1) * H],
            "keys": keys[c * H : (c + 1) * H],
            "values": values[c * H : (c + 1) * H],
        }
        for c in range(N_CORES)
    ]
    res = run_bass_kernel_spmd(nc, in_maps, core_ids=list(range(N_CORES)))
    return np.concatenate([r["out"] for r in res.results], axis=0)
